# revision 15
# baseline (speedup 1.0000x reference)
"""Trainium2 Bass kernel for nn_Attention (dot-product attention summary).

reference:
    scores[b,s] = <data[b,s,:], crit[b,:]>       # [B, S]
    weights     = softmax(scores, axis=-1)
    summary[b]  = sum_s weights[b,s] * data[b,s] # [B, D]

Sharding: B=8 batches -> one batch per NeuronCore (pure data parallel, no
collectives). Per core: data [S=4096, D=1024] f32 (16.8 MB), crit [D].

Single HBM pass per core:
  - data cast-DMA'd (gpsimd/SWDGE) to SBUF as float32r (PE fast path;
    ~2.4e-4 elementwise rounding, harmless here).
  - pass 1 (scores): DVE tensor_tensor_reduce per 128-row chunk against a
    broadcast crit tile.
  - softmax: G groups; per-group cross-partition max (DVE free-reduce +
    gpsimd partition_all_reduce), flash-style running max with ACT
    in-place PSUM rescale between groups (verified: ACT writes preserve
    PSUM has_written, so PE keeps accumulating).
  - pass 2: PE f32r matmuls (lhsT = exp-weight column, rhs = data chunk)
    into one PSUM pair [1,512]x2.
  - tail: Z from per-group z columns * exp(M_g - M_final), reciprocal,
    scaled copy to SBUF, one DMA out.

Toolchain constraint: walrus accepts at most ONE semaphore wait per
instruction and Tile does not split waits. Absorber ops keep every
instruction at <=1 new semaphore; an SP reg_load chain at the end absorbs
all outstanding sems so the auto-emitted drain fits the limit.
"""

import numpy as np
from contextlib import ExitStack

import concourse.bass as bass
import concourse.bass_isa as bass_isa
import concourse.tile as tile
from concourse import mybir
from concourse.bass import _add_dep_helper
from concourse.bass_utils import run_bass_kernel_spmd

B, S, D = 8, 4096, 1024
P = 128                 # partitions
NT = 8                  # DMA tiles
CPT = S // P // NT      # chunks per tile = 4
NCHUNK = S // P         # 32 chunks of 128 rows
G = 4                   # softmax groups
GB = [0, 10, 20, 28, 32]  # group chunk bounds (small last group -> short tail)
CPG = NCHUNK // G       # legacy (unused in loop)
F32 = mybir.dt.float32
F32R = mybir.dt.float32r

_NC_CACHE = None


def build():
    nc = bass.Bass()
    data_ext = nc.declare_dram_parameter("data", [S, D], F32, isOutput=False)
    crit_ext = nc.declare_dram_parameter("crit", [1, D], F32, isOutput=False)
    out_ext = nc.declare_dram_parameter("out", [1, D], F32, isOutput=True)

    dmas = []     # DMA instruction handles for the absorption tail
    with tile.TileContext(nc) as tc, ExitStack() as ctx:
        sb = ctx.enter_context(tc.tile_pool(name="sb", bufs=1))
        ps = ctx.enter_context(tc.tile_pool(name="ps", bufs=1, space="PSUM"))

        # ---- inputs -------------------------------------------------------
        crit_b = sb.tile([P, D], F32)
        dmas.append(nc.sync.dma_start(crit_b, crit_ext[:].to_broadcast([P, D])))

        # Row permutation s = 512*t + 4*p + j makes each partition's bytes
        # 16KB-contiguous (4x larger DMA descriptors -> ~390 GB/s vs ~330).
        # softmax+sum over S are order-invariant, so any fixed permutation
        # is fine as long as scores and pass-2 use the same chunk mapping.
        dtiles = []
        dview = data_ext[:].rearrange("(t p j) d -> t p (j d)", p=P, j=CPT)
        for t in range(NT):
            dt_ = sb.tile([P, CPT * D], F32R, tag=f"dt{t}")
            dmas.append(nc.gpsimd.dma_start(dt_, dview[t]))
            dtiles.append(dt_)

        # constants: identity (for PE transpose) and a ones row (broadcast)
        ident = sb.tile([P, P], F32)
        nc.gpsimd.memset(ident, 0.0)
        last_gps = nc.gpsimd.affine_select(
            out=ident, in_=ident, compare_op=mybir.AluOpType.not_equal,
            fill=1.0, base=0, pattern=[[-1, P]], channel_multiplier=1)
        ones_row = sb.tile([1, P], F32)
        nc.gpsimd.memset(ones_row, 1.0)
        ones_col = sb.tile([P, 1], F32)
        last_gps = nc.gpsimd.memset(ones_col, 1.0)

        # early SP absorbers: observe each input-DMA lane as it completes
        scrapc = sb.tile([1, 1], mybir.dt.int32)
        nc.sync.store(scrapc[0:1, 0:1], 0)
        areg = nc.sync.alloc_register("absorb")
        nc.sync.reg_load(areg, scrapc[0:1, 0:1])  # absorb SP_sequencer RAW
        for t_ in dmas:
            ld = nc.sync.reg_load(areg, scrapc[0:1, 0:1])
            _add_dep_helper(ld.ins, t_.ins, sync=True, reason="wait-split absorber")
        early_absorbed = list(dmas)

        # warm the ACT exp table early (one-time ~2.7us load)
        warm = sb.tile([1, 2], F32)
        nc.vector.memset(warm, 0.0)
        last_act = nc.scalar.activation(
            warm, warm, mybir.ActivationFunctionType.Exp)

        # ---- state --------------------------------------------------------
        scores = sb.tile([P, NCHUNK], F32)
        prod = sb.tile([P, D], F32)          # ttr mandatory elementwise out
        dve_scr = sb.tile([1, NT + 2], F32)  # per-tile DVE lane absorbers
        mloc = sb.tile([P, G], F32)
        mall = sb.tile([P, G], F32)
        dtmp = sb.tile([P, G], F32)
        mbuf = sb.tile([P, G], F32)          # running max after each group
        zbuf = sb.tile([P, G], F32)          # per-group z partial sums
        negm = sb.tile([P, G], F32)
        rtile = sb.tile([P, G], F32)         # group rescale factors (g>=1)
        wbuf = sb.tile([P, NCHUNK], F32R)    # exp weights (f32r for PE)
        act_scr = sb.tile([1, G + 2], F32)   # ACT psum observers

        a_lo = ps.tile([1, 512], F32, tag="a_lo")
        a_hi = ps.tile([1, 512], F32, tag="a_hi")
        tp_ps = ps.tile([1, P], F32, tag="tp_ps")
        bc_ps = ps.tile([P, 1], F32, tag="bc_ps")
        mgs = sb.tile([1, G], F32, tag="mgs")
        pe_scr_t = ps.tile([1, 2], F32, tag="pe_scr")
        pe_scr = [pe_scr_t] * G

        # absorber: first DVE touch of crit_b
        nc.vector.tensor_copy(dve_scr[0:1, NT : NT + 1], crit_b[0:1, 0:1])
        # early PE absorber: observe Pool (ident/ones consts) once
        pe_boot = nc.tensor.matmul(
            pe_scr[0], ident[:, 0:1], ident[:, 0:2], start=True, stop=True)
        _add_dep_helper(pe_boot.ins, last_gps.ins, sync=True,
                        reason="PE observes latest Pool const tick")

        last_pe = None
        prev_chain_end = None
        for g in range(G):
            c_lo, c_hi = GB[g], GB[g + 1]
            first_stt = None
            # DVE lane absorbers on first touch of each tile, then scores
            for c in range(c_lo, c_hi):
                t, j = c // CPT, c % CPT
                if j == 0:
                    nc.vector.tensor_copy(
                        dve_scr[0:1, t : t + 1],
                        dtiles[t][0:1, 0:1].bitcast(F32))
                stt = nc.vector.scalar_tensor_tensor(
                    out=prod,
                    in0=dtiles[t][:, j * D : (j + 1) * D].bitcast(F32),
                    scalar=1.0,
                    in1=crit_b,
                    op0=mybir.AluOpType.mult,
                    op1=mybir.AluOpType.mult,
                    accum_out=scores[:, c : c + 1],
                )
                if first_stt is None:
                    first_stt = stt
            if prev_chain_end is not None:
                # keep the previous group's softmax chain INLINE in the DVE
                # stream (scheduler otherwise defers all chains past all
                # scoring, serializing exp+pass-2 into a long tail)
                _add_dep_helper(first_stt.ins, prev_chain_end.ins, sync=False,
                                reason="inline group chain before next scores")
            # group max -> all partitions
            nc.vector.tensor_reduce(
                out=mloc[:, g : g + 1], in_=scores[:, c_lo:c_hi],
                axis=mybir.AxisListType.XYZW, op=mybir.AluOpType.max)
            # cross-partition max: PE transpose -> DVE reduce -> PE bcast
            nc.tensor.matmul(tp_ps, mloc[:, g : g + 1], ident,
                             start=True, stop=True)
            nc.vector.reduce_max(mgs[0:1, g : g + 1], tp_ps,
                                 axis=mybir.AxisListType.XYZW)
            nc.tensor.matmul(bc_ps, ones_row, mgs[0:1, g : g + 1],
                             start=True, stop=True)
            nc.vector.tensor_copy(mall[:, g : g + 1], bc_ps)
            if g == 0:
                nc.vector.tensor_copy(mbuf[:, 0:1], mall[:, 0:1])
            else:
                # d = min(M_prev - m_g, 0) ; M_g = max(M_prev, m_g)
                nc.vector.tensor_sub(
                    dtmp[:, g : g + 1], mbuf[:, g - 1 : g], mall[:, g : g + 1])
                nc.vector.tensor_scalar_min(
                    dtmp[:, g : g + 1], dtmp[:, g : g + 1], 0.0)
                nc.vector.tensor_max(
                    mbuf[:, g : g + 1], mbuf[:, g - 1 : g], mall[:, g : g + 1])
            prev_chain_end = nc.vector.tensor_scalar_mul(
                negm[:, g : g + 1], mbuf[:, g : g + 1], -1.0)
            if g > 0:
                # r_g = exp(d)
                nc.scalar.activation(
                    rtile[:, g : g + 1], dtmp[:, g : g + 1],
                    mybir.ActivationFunctionType.Exp)
            # w_g = exp(scores_g - M_g), z_g = rowsum(w_g)
            last_act = nc.scalar.activation(
                out=wbuf[:, c_lo:c_hi],
                in_=scores[:, c_lo:c_hi],
                func=mybir.ActivationFunctionType.Exp,
                bias=negm[:, g : g + 1],
                scale=1.0,
                accum_out=zbuf[:, g : g + 1],
            )
            resc_hi = None
            if g > 0:
                # observe PE on ACT, then rescale running psum by r_g
                nc.scalar.copy(act_scr[0:1, g : g + 1], a_lo[0:1, 0:1])
                nc.scalar.mul(a_lo, a_lo, rtile[0:1, g : g + 1])
                resc_hi = last_act = nc.scalar.mul(a_hi, a_hi, rtile[0:1, g : g + 1])
            # PE absorber AFTER the rescales: pin it to the latest ACT tick
            c0 = c_lo
            pe_abs = nc.tensor.matmul(
                pe_scr[g], wbuf[:, c0 : c0 + 1], wbuf[:, c0 : c0 + 2],
                start=True, stop=True)
            if resc_hi is not None:
                _add_dep_helper(pe_abs.ins, resc_hi.ins, sync=True,
                                reason="absorb latest ACT tick before psum matmuls")
            for c in range(c_lo, c_hi):
                t, j = c // CPT, c % CPT
                mm_lo = nc.tensor.matmul(
                    a_lo, wbuf[:, c : c + 1], dtiles[t][:, j * D : j * D + 512],
                    start=(c == 0), stop=(c == NCHUNK - 1))
                if c == c_lo:
                    _add_dep_helper(mm_lo.ins, pe_abs.ins, sync=True,
                                    reason="order first group matmul after absorber")
                last_pe = nc.tensor.matmul(
                    a_hi, wbuf[:, c : c + 1],
                    dtiles[t][:, j * D + 512 : (j + 1) * D],
                    start=(c == 0), stop=(c == NCHUNK - 1))

        # ---- tail ---------------------------------------------------------
        # f_all[:, g] = exp(M_g - M_final);  zfin = sum_g f_g * zbuf[:, g]
        negmf = sb.tile([P, 1], F32)
        nc.vector.tensor_scalar_mul(negmf, mbuf[:, G - 1 : G], -1.0)
        f_all = sb.tile([P, G], F32)
        last_act = nc.scalar.activation(
            f_all, mbuf, mybir.ActivationFunctionType.Exp, bias=negmf)
        fscr = sb.tile([P, G], F32)
        zfin = sb.tile([P, 1], F32)
        nc.vector.scalar_tensor_tensor(
            out=fscr, in0=zbuf, scalar=1.0, in1=f_all,
            op0=mybir.AluOpType.mult, op1=mybir.AluOpType.mult,
            accum_out=zfin)
        # Z = sum_p zfin[p]: matmul with ones column -> [1,1]
        zsc = ps.tile([1, 1], F32, tag="zsc")
        nc.tensor.matmul(zsc, zfin, ones_col, start=True, stop=True)
        zall = sb.tile([1, 1], F32)
        nc.vector.tensor_copy(zall, zsc)
        recip = sb.tile([1, 1], F32)
        last_dve = nc.vector.reciprocal(recip, zall)

        out_sb = sb.tile([1, D], F32)
        # absorber on ACT: observe DVE's recip before touching PSUM (PE sem)
        nc.scalar.copy(act_scr[0:1, G : G + 1], recip)
        nc.scalar.copy(act_scr[0:1, G + 1 : G + 2], a_lo[0:1, 0:1])
        nc.scalar.mul(out_sb[:, 0:512], a_lo, recip)
        last_act = nc.scalar.mul(out_sb[:, 512:1024], a_hi, recip)
        dmas.append(nc.sync.dma_start(out_ext[:], out_sb))

        # ---- absorption tail: SP observes remaining outstanding sems ------
        for t in [x for x in dmas if x not in early_absorbed] + [
                last_pe, last_act, last_dve, last_gps]:
            ld = nc.sync.reg_load(areg, scrapc[0:1, 0:1])
            _add_dep_helper(ld.ins, t.ins, sync=True, reason="wait-split absorber")
        nc.sync.free_register(areg)

    return nc


LAST_EXEC_NS = None


def kernel(data: np.ndarray, crit: np.ndarray) -> np.ndarray:
    global _NC_CACHE, LAST_EXEC_NS
    if _NC_CACHE is None:
        _NC_CACHE = build()
    nc = _NC_CACHE
    data = np.ascontiguousarray(data, dtype=np.float32)
    crit = np.ascontiguousarray(crit, dtype=np.float32)
    in_maps = [
        {"data": data[b], "crit": crit[b : b + 1]} for b in range(B)
    ]
    import os
    trace = bool(os.environ.get("BASS_KERNEL_TRACE"))
    res = run_bass_kernel_spmd(nc, in_maps, list(range(B)), trace=trace)
    LAST_EXEC_NS = res.exec_time_ns
    out = np.stack([res.results[b]["out"][0] for b in range(B)])
    return out.astype(np.float32)


if __name__ == "__main__":
    rng = np.random.default_rng(0)
    d = rng.standard_normal((B, S, D), dtype=np.float32)
    c = rng.standard_normal((B, D), dtype=np.float32)
    o = kernel(d, c)
    sc = np.einsum("bsd,bd->bs", d, c)
    w = np.exp(sc - sc.max(-1, keepdims=True))
    w /= w.sum(-1, keepdims=True)
    ref = np.einsum("bs,bsd->bd", w, d)
    rel = np.linalg.norm(o - ref) / np.linalg.norm(ref)
    print("rel err:", rel)


# revision 16
# speedup vs baseline: 1.2201x; 1.2201x over previous
"""Trainium2 Bass kernel for nn_Attention (dot-product attention summary).

reference:
    scores[b,s] = <data[b,s,:], crit[b,:]>       # [B, S]
    weights     = softmax(scores, axis=-1)
    summary[b]  = sum_s weights[b,s] * data[b,s] # [B, D]

Sharding: B=8 batches -> one batch per NeuronCore (pure data parallel, no
collectives). Per core: data [S=4096, D=1024] f32 (16.8 MB), crit [D].

Single HBM pass per core:
  - data cast-DMA'd (gpsimd/SWDGE) to SBUF as float32r (PE fast path;
    ~2.4e-4 elementwise rounding, harmless here).
  - pass 1 (scores): DVE tensor_tensor_reduce per 128-row chunk against a
    broadcast crit tile.
  - softmax: G groups; per-group cross-partition max (DVE free-reduce +
    gpsimd partition_all_reduce), flash-style running max with ACT
    in-place PSUM rescale between groups (verified: ACT writes preserve
    PSUM has_written, so PE keeps accumulating).
  - pass 2: PE f32r matmuls (lhsT = exp-weight column, rhs = data chunk)
    into one PSUM pair [1,512]x2.
  - tail: Z from per-group z columns * exp(M_g - M_final), reciprocal,
    scaled copy to SBUF, one DMA out.

Toolchain constraint: walrus accepts at most ONE semaphore wait per
instruction and Tile does not split waits. Absorber ops keep every
instruction at <=1 new semaphore; an SP reg_load chain at the end absorbs
all outstanding sems so the auto-emitted drain fits the limit.
"""

import numpy as np
from contextlib import ExitStack

import concourse.bass as bass
import concourse.bass_isa as bass_isa
import concourse.tile as tile
from concourse import mybir
from concourse.bass import _add_dep_helper
from concourse.bass_utils import run_bass_kernel_spmd

B, S, D = 8, 4096, 1024
P = 128                 # partitions
NT = 8                  # DMA tiles
CPT = S // P // NT      # chunks per tile = 4
NCHUNK = S // P         # 32 chunks of 128 rows
G = 4                   # softmax groups
GB = [0, 10, 20, 28, 32]  # group chunk bounds (small last group -> short tail)
CPG = NCHUNK // G       # legacy (unused in loop)
F32 = mybir.dt.float32
F32R = mybir.dt.float32r

_NC_CACHE = None


def build():
    nc = bass.Bass()
    data_ext = nc.declare_dram_parameter("data", [S, D], F32, isOutput=False)
    crit_ext = nc.declare_dram_parameter("crit", [1, D], F32, isOutput=False)
    cb_ext = nc.declare_dram_parameter("cb", [P, P + 1], F32, isOutput=False)
    orow_ext = nc.declare_dram_parameter("orow", [1, P], F32, isOutput=False)
    out_ext = nc.declare_dram_parameter("out", [1, D], F32, isOutput=True)

    dmas = []     # DMA instruction handles for the absorption tail
    with tile.TileContext(nc) as tc, ExitStack() as ctx:
        sb = ctx.enter_context(tc.tile_pool(name="sb", bufs=1))
        ps = ctx.enter_context(tc.tile_pool(name="ps", bufs=1, space="PSUM"))

        # ---- inputs -------------------------------------------------------
        crit_b = sb.tile([P, D], F32)
        dmas.append(nc.sync.dma_start(crit_b, crit_ext[:].to_broadcast([P, D])))

        # Row permutation s = 512*t + 4*p + j makes each partition's bytes
        # 16KB-contiguous (4x larger DMA descriptors -> ~390 GB/s vs ~330).
        # softmax+sum over S are order-invariant, so any fixed permutation
        # is fine as long as scores and pass-2 use the same chunk mapping.
        dtiles = []
        dview = data_ext[:].rearrange("(t p j) d -> t p (j d)", p=P, j=CPT)
        for t in range(NT):
            dt_ = sb.tile([P, CPT * D], F32R, tag=f"dt{t}")
            dmas.append(nc.gpsimd.dma_start(dt_, dview[t]))
            dtiles.append(dt_)

        # constants from host (identity | ones-col, and a ones row):
        # building them with gpsimd ops would queue behind ~48us of SWDGE
        # descriptor emission on the Pool sequencer.
        cbt = sb.tile([P, P + 1], F32)
        cb_dma = nc.sync.dma_start(cbt, cb_ext[:])
        dmas.append(cb_dma)
        orow_t = sb.tile([1, P], F32)
        orow_dma = nc.sync.dma_start(orow_t, orow_ext[:])
        dmas.append(orow_dma)
        ident = cbt[:, 0:P]
        ones_col = cbt[:, P : P + 1]
        ones_row = orow_t[:]

        # early SP absorbers: observe each input-DMA lane as it completes
        scrapc = sb.tile([1, 1], mybir.dt.int32)
        nc.sync.store(scrapc[0:1, 0:1], 0)
        areg = nc.sync.alloc_register("absorb")
        nc.sync.reg_load(areg, scrapc[0:1, 0:1])  # absorb SP_sequencer RAW
        for t_ in dmas:
            ld = nc.sync.reg_load(areg, scrapc[0:1, 0:1])
            _add_dep_helper(ld.ins, t_.ins, sync=True, reason="wait-split absorber")
        early_absorbed = list(dmas)

        # warm the ACT exp table early (one-time ~2.7us load)
        warm = sb.tile([1, 2], F32)
        nc.vector.memset(warm, 0.0)
        last_act = nc.scalar.activation(
            warm, warm, mybir.ActivationFunctionType.Exp)

        # ---- state --------------------------------------------------------
        scores = sb.tile([P, NCHUNK], F32)
        prod = sb.tile([P, D], F32)          # ttr mandatory elementwise out
        dve_scr = sb.tile([1, NT + 2], F32)  # per-tile DVE lane absorbers
        mloc = sb.tile([P, G], F32)
        mall = sb.tile([P, G], F32)
        dtmp = sb.tile([P, G], F32)
        mbuf = sb.tile([P, G], F32)          # running max after each group
        zbuf = sb.tile([P, G], F32)          # per-group z partial sums
        negm = sb.tile([P, G], F32)
        rtile = sb.tile([P, G], F32)         # group rescale factors (g>=1)
        wbuf = sb.tile([P, NCHUNK], F32R)    # exp weights (f32r for PE)
        act_scr = sb.tile([1, G + 2], F32)   # ACT psum observers

        a_lo = ps.tile([1, 512], F32, tag="a_lo")
        a_hi = ps.tile([1, 512], F32, tag="a_hi")
        tp_ps = ps.tile([1, P], F32, tag="tp_ps")
        bc_ps = ps.tile([P, 1], F32, tag="bc_ps")
        mgs = sb.tile([1, G], F32, tag="mgs")
        pe_scr_t = ps.tile([P, 2], F32, tag="pe_scr")
        pe_scr = [pe_scr_t] * G

        # absorber: first DVE touch of crit_b
        nc.vector.tensor_copy(dve_scr[0:1, NT : NT + 1], crit_b[0:1, 0:1])
        # early PE absorbers: observe the two const-DMA lanes
        nc.tensor.matmul(
            pe_scr[0][0:1, :], ident[:, 0:1], ident[:, 0:2],
            start=True, stop=True)
        nc.tensor.matmul(
            pe_scr[0], ones_row, ones_row[0:1, 0:2], start=True, stop=True)

        last_pe = None
        prev_chain_end = None
        for g in range(G):
            c_lo, c_hi = GB[g], GB[g + 1]
            first_stt = None
            # DVE lane absorbers on first touch of each tile, then scores
            for c in range(c_lo, c_hi):
                t, j = c // CPT, c % CPT
                if j == 0:
                    nc.vector.tensor_copy(
                        dve_scr[0:1, t : t + 1],
                        dtiles[t][0:1, 0:1].bitcast(F32))
                stt = nc.vector.scalar_tensor_tensor(
                    out=prod,
                    in0=dtiles[t][:, j * D : (j + 1) * D].bitcast(F32),
                    scalar=1.0,
                    in1=crit_b,
                    op0=mybir.AluOpType.mult,
                    op1=mybir.AluOpType.mult,
                    accum_out=scores[:, c : c + 1],
                )
                if first_stt is None:
                    first_stt = stt
            if prev_chain_end is not None:
                # keep the previous group's softmax chain INLINE in the DVE
                # stream (scheduler otherwise defers all chains past all
                # scoring, serializing exp+pass-2 into a long tail)
                _add_dep_helper(first_stt.ins, prev_chain_end.ins, sync=False,
                                reason="inline group chain before next scores")
            # group max -> all partitions
            nc.vector.tensor_reduce(
                out=mloc[:, g : g + 1], in_=scores[:, c_lo:c_hi],
                axis=mybir.AxisListType.XYZW, op=mybir.AluOpType.max)
            # cross-partition max: PE transpose -> DVE reduce -> PE bcast
            nc.tensor.matmul(tp_ps, mloc[:, g : g + 1], ident,
                             start=True, stop=True)
            nc.vector.reduce_max(mgs[0:1, g : g + 1], tp_ps,
                                 axis=mybir.AxisListType.XYZW)
            nc.tensor.matmul(bc_ps, ones_row, mgs[0:1, g : g + 1],
                             start=True, stop=True)
            nc.vector.tensor_copy(mall[:, g : g + 1], bc_ps)
            if g == 0:
                nc.vector.tensor_copy(mbuf[:, 0:1], mall[:, 0:1])
            else:
                # d = min(M_prev - m_g, 0) ; M_g = max(M_prev, m_g)
                nc.vector.tensor_sub(
                    dtmp[:, g : g + 1], mbuf[:, g - 1 : g], mall[:, g : g + 1])
                nc.vector.tensor_scalar_min(
                    dtmp[:, g : g + 1], dtmp[:, g : g + 1], 0.0)
                nc.vector.tensor_max(
                    mbuf[:, g : g + 1], mbuf[:, g - 1 : g], mall[:, g : g + 1])
            prev_chain_end = nc.vector.tensor_scalar_mul(
                negm[:, g : g + 1], mbuf[:, g : g + 1], -1.0)
            if g > 0:
                # r_g = exp(d)
                nc.scalar.activation(
                    rtile[:, g : g + 1], dtmp[:, g : g + 1],
                    mybir.ActivationFunctionType.Exp)
            # w_g = exp(scores_g - M_g), z_g = rowsum(w_g)
            last_act = nc.scalar.activation(
                out=wbuf[:, c_lo:c_hi],
                in_=scores[:, c_lo:c_hi],
                func=mybir.ActivationFunctionType.Exp,
                bias=negm[:, g : g + 1],
                scale=1.0,
                accum_out=zbuf[:, g : g + 1],
            )
            resc_hi = None
            if g > 0:
                # observe PE on ACT, then rescale running psum by r_g
                nc.scalar.copy(act_scr[0:1, g : g + 1], a_lo[0:1, 0:1])
                nc.scalar.mul(a_lo, a_lo, rtile[0:1, g : g + 1])
                resc_hi = last_act = nc.scalar.mul(a_hi, a_hi, rtile[0:1, g : g + 1])
            # PE absorber AFTER the rescales: pin it to the latest ACT tick
            c0 = c_lo
            pe_abs = nc.tensor.matmul(
                pe_scr[g][0:1, :], wbuf[:, c0 : c0 + 1], wbuf[:, c0 : c0 + 2],
                start=True, stop=True)
            if resc_hi is not None:
                _add_dep_helper(pe_abs.ins, resc_hi.ins, sync=True,
                                reason="absorb latest ACT tick before psum matmuls")
            for c in range(c_lo, c_hi):
                t, j = c // CPT, c % CPT
                mm_lo = nc.tensor.matmul(
                    a_lo, wbuf[:, c : c + 1], dtiles[t][:, j * D : j * D + 512],
                    start=(c == 0), stop=(c == NCHUNK - 1))
                if c == c_lo:
                    _add_dep_helper(mm_lo.ins, pe_abs.ins, sync=True,
                                    reason="order first group matmul after absorber")
                last_pe = nc.tensor.matmul(
                    a_hi, wbuf[:, c : c + 1],
                    dtiles[t][:, j * D + 512 : (j + 1) * D],
                    start=(c == 0), stop=(c == NCHUNK - 1))

        # ---- tail ---------------------------------------------------------
        # f_all[:, g] = exp(M_g - M_final);  zfin = sum_g f_g * zbuf[:, g]
        negmf = sb.tile([P, 1], F32)
        nc.vector.tensor_scalar_mul(negmf, mbuf[:, G - 1 : G], -1.0)
        f_all = sb.tile([P, G], F32)
        last_act = nc.scalar.activation(
            f_all, mbuf, mybir.ActivationFunctionType.Exp, bias=negmf)
        fscr = sb.tile([P, G], F32)
        zfin = sb.tile([P, 1], F32)
        nc.vector.scalar_tensor_tensor(
            out=fscr, in0=zbuf, scalar=1.0, in1=f_all,
            op0=mybir.AluOpType.mult, op1=mybir.AluOpType.mult,
            accum_out=zfin)
        # Z = sum_p zfin[p]: matmul with ones column -> [1,1]
        zsc = ps.tile([1, 1], F32, tag="zsc")
        nc.tensor.matmul(zsc, zfin, ones_col, start=True, stop=True)
        zall = sb.tile([1, 1], F32)
        nc.vector.tensor_copy(zall, zsc)
        recip = sb.tile([1, 1], F32)
        last_dve = nc.vector.reciprocal(recip, zall)

        out_sb = sb.tile([1, D], F32)
        # absorber on ACT: observe DVE's recip before touching PSUM (PE sem)
        nc.scalar.copy(act_scr[0:1, G : G + 1], recip)
        nc.scalar.copy(act_scr[0:1, G + 1 : G + 2], a_lo[0:1, 0:1])
        nc.scalar.mul(out_sb[:, 0:512], a_lo, recip)
        last_act = nc.scalar.mul(out_sb[:, 512:1024], a_hi, recip)
        dmas.append(nc.sync.dma_start(out_ext[:], out_sb))

        # ---- absorption tail: SP observes remaining outstanding sems ------
        for t in [x for x in dmas if x not in early_absorbed] + [
                last_pe, last_act, last_dve]:
            ld = nc.sync.reg_load(areg, scrapc[0:1, 0:1])
            _add_dep_helper(ld.ins, t.ins, sync=True, reason="wait-split absorber")
        nc.sync.free_register(areg)

    return nc


LAST_EXEC_NS = None


def kernel(data: np.ndarray, crit: np.ndarray) -> np.ndarray:
    global _NC_CACHE, LAST_EXEC_NS
    if _NC_CACHE is None:
        _NC_CACHE = build()
    nc = _NC_CACHE
    data = np.ascontiguousarray(data, dtype=np.float32)
    crit = np.ascontiguousarray(crit, dtype=np.float32)
    cb = np.concatenate(
        [np.eye(P, dtype=np.float32), np.ones((P, 1), np.float32)], axis=1)
    orow = np.ones((1, P), np.float32)
    in_maps = [
        {"data": data[b], "crit": crit[b : b + 1], "cb": cb, "orow": orow}
        for b in range(B)
    ]
    import os
    trace = bool(os.environ.get("BASS_KERNEL_TRACE"))
    res = run_bass_kernel_spmd(nc, in_maps, list(range(B)), trace=trace)
    LAST_EXEC_NS = res.exec_time_ns
    out = np.stack([res.results[b]["out"][0] for b in range(B)])
    return out.astype(np.float32)


if __name__ == "__main__":
    rng = np.random.default_rng(0)
    d = rng.standard_normal((B, S, D), dtype=np.float32)
    c = rng.standard_normal((B, D), dtype=np.float32)
    o = kernel(d, c)
    sc = np.einsum("bsd,bd->bs", d, c)
    w = np.exp(sc - sc.max(-1, keepdims=True))
    w /= w.sum(-1, keepdims=True)
    ref = np.einsum("bs,bsd->bd", w, d)
    rel = np.linalg.norm(o - ref) / np.linalg.norm(ref)
    print("rel err:", rel)
